# revision 5
# baseline (speedup 1.0000x reference)
"""Multi-head attention (B=2, N=4096, D=512, H=8) on 8 TRN2 NeuronCores.

Sharding: head-parallel. Core d owns head d for both batches:
  - QKV: tensor-parallel slices of w_qkv (per-head 64-dim slices), computed
    from a replicated transposed activation xT = x.T (bf16).
  - Attention: flash-style, scores kept transposed (S.T = k @ q.T per
    128-k-token tile), softmax without max subtraction (scores ~ N(0,1)),
    exp on ScalarE with the 1/sqrt(hd) scale fused in, attn.T @ v via a
    [v | ones] stationary operand so the softmax denominator falls out of
    the same matmul (row 0 of the accumulator).
  - AllToAll redistributes normalized per-head outputs so core d holds all
    heads for tokens [d*1024, (d+1)*1024), then a local output projection
    (bias folded in as a K=1 matmul term) produces that token slice.
Host side only transposes/casts inputs and concatenates the 8 output slices.
"""

import numpy as np
import ml_dtypes

N_CORES = 8
B, N, D = 2, 4096, 512
H, HD = 8, 64
T = B * N              # 8192 flattened tokens
TS = T // N_CORES      # 1024 tokens output slice per core
SCALE = HD ** -0.5
KC = D // 128          # 4 contraction chunks of the model dim
NKT = N // 128         # 32 k-token tiles per batch
QC = 1024              # q-chunk (columns) processed per accumulator
NQC = N // QC          # 4 q-chunks per batch

BF16 = ml_dtypes.bfloat16

_COMPILED = {}


def _patch_tile_drain():
    """The walrus build in this container caps sync waits at 1 per
    instruction (2 for EventSemaphore), but TileContext._drain_and_barrier
    puts every live proc's final wait on a single Drain, which fails
    codegen with 'Too many sync wait commands'. Re-emit those waits as
    individual wait_ge instructions before the drain."""
    import concourse.mybir as mybir
    import concourse.tile as tile
    from concourse.bass_types import SemaphoreHandle
    from concourse.vector_clock import ScopedClock

    if getattr(tile.TileContext, "_drain_patch_installed", False):
        return

    def _drain_and_barrier(self, tick_clock, wait_clock):
        probe = mybir.InstNoOp(name=f"drain-probe-{self.nc.next_id()}", ins=[], outs=[])
        probe.engine = mybir.EngineType.SP
        wait_clock.add_sem_waits(probe, ScopedClock({None: tick_clock.global_clock}))
        waits = probe.sync_info.on_wait if probe.sync_info is not None else []
        for w in waits:
            assert w.wait_mode == "sem-ge-imm", w
            self.nc.sync.wait_ge(SemaphoreHandle(w.ant_name, w.id), w.wait_value)
        self.nc.sync.drain()

        self.nc.all_engine_barrier()
        assert self.sems is not None
        popped = self.nc._tile_sem_poison_stack.pop()
        assert popped is self._sem_poison
        self.nc.clear_and_free_semaphores(list(self.sems.allocated().values()))
        self.nc.all_engine_barrier()

    tile.TileContext._drain_and_barrier = _drain_and_barrier
    tile.TileContext._drain_patch_installed = True


def _patch_multiwait_split():
    """This walrus build rejects instructions with more than one sync wait
    ('Too many sync wait commands'), but Tile's wait assigner can emit
    several waits on one instruction. Post-process the serialized BIR:
    move excess waits onto single-wait EventSemaphore instructions inserted
    just before the owning instruction (same engine => executes in order)."""
    import json

    import concourse.bass as bass

    if getattr(bass.Bass, "_multiwait_patch_installed", False):
        return
    orig = bass.Bass.to_json_bytes

    def to_json_bytes(self, *a, **kw):
        data = json.loads(orig(self, *a, **kw))
        n_split = 0
        for fn in data.get("functions", []):
            for bb in fn.get("blocks", []):
                insts = bb.get("instructions")
                if not insts:
                    continue
                out = []
                for inst in insts:
                    si = inst.get("sync_info")
                    ow = (si or {}).get("on_wait") or []
                    if len(ow) > 1:
                        for i, w in enumerate(ow[:-1]):
                            out.append({
                                "debug": inst.get("debug", 0),
                                "engine": inst["engine"],
                                "ins": [],
                                "outs": [],
                                "name": f"{inst['name']}-esw{i}",
                                "opcode": "EventSemaphore",
                                "sync_info": {"on_update": [], "on_wait": [w]},
                            })
                            n_split += 1
                        si["on_wait"] = [ow[-1]]
                    out.append(inst)
                bb["instructions"] = out
        return json.dumps(data).encode()

    bass.Bass.to_json_bytes = to_json_bytes
    bass.Bass._multiwait_patch_installed = True


def _build():
    from contextlib import ExitStack

    import concourse.bass as bass
    import concourse.mybir as mybir
    import concourse.tile as tile

    _patch_tile_drain()
    _patch_multiwait_split()
    dt = mybir.dt
    nc = bass.Bass(num_devices=N_CORES)

    xT_ext = nc.declare_dram_parameter("xT", [D, T], dt.bfloat16, isOutput=False)
    wqT_ext = nc.declare_dram_parameter("wqT", [D, HD], dt.bfloat16, isOutput=False)
    wkT_ext = nc.declare_dram_parameter("wkT", [D, HD], dt.bfloat16, isOutput=False)
    wvT_ext = nc.declare_dram_parameter("wvT", [D, HD], dt.bfloat16, isOutput=False)
    wpT_ext = nc.declare_dram_parameter("wpT", [D, D], dt.bfloat16, isOutput=False)
    bias_ext = nc.declare_dram_parameter("bias", [1, D], dt.bfloat16, isOutput=False)
    out_ext = nc.declare_dram_parameter("out", [TS, D], dt.float32, isOutput=True)

    with tile.TileContext(nc) as tc, ExitStack() as ctx:
        singles = ctx.enter_context(tc.tile_pool(name="singles", bufs=1))
        dram = ctx.enter_context(tc.tile_pool(name="dram", bufs=2, space="DRAM"))
        cpool = ctx.enter_context(tc.tile_pool(name="cpool", bufs=4))

        # ---------- persistent SBUF ----------
        xT_sb = singles.tile([128, KC, T], dt.bfloat16)        # 64 KB/part
        wqT_sb = singles.tile([128, KC, HD], dt.bfloat16)
        wkT_sb = singles.tile([128, KC, HD], dt.bfloat16)
        wvT_sb = singles.tile([128, KC, HD], dt.bfloat16)
        wpT_sb = singles.tile([128, KC, D], dt.bfloat16)
        bias_sb = singles.tile([1, D], dt.bfloat16)
        ones_sb = singles.tile([1, 128], dt.bfloat16)
        qT_sb = singles.tile([64, T], dt.bfloat16)             # 16 KB/part
        kT_sb = singles.tile([64, T], dt.bfloat16)
        vp_sb = singles.tile([128, T // 128, 1 + HD], dt.bfloat16)  # [ones | v]
        outTall_sb = singles.tile([128, KC, TS], dt.bfloat16)

        a2a_in = dram.tile([N_CORES, HD, TS], dt.bfloat16)
        a2a_out = dram.tile([N_CORES, HD, TS], dt.bfloat16)

        for k in range(KC):
            nc.sync.dma_start(
                out=xT_sb[:, k, :], in_=xT_ext[k * 128:(k + 1) * 128, :]
            )
        for w_sb, w_ext in ((wqT_sb, wqT_ext), (wkT_sb, wkT_ext), (wvT_sb, wvT_ext)):
            nc.sync.dma_start(
                out=w_sb[:], in_=w_ext[:].rearrange("(k p) c -> p k c", p=128)
            )
        nc.sync.dma_start(
            out=wpT_sb[:], in_=wpT_ext[:].rearrange("(k p) c -> p k c", p=128)
        )
        nc.sync.dma_start(out=bias_sb[:], in_=bias_ext[:])
        nc.vector.memset(ones_sb[:], 1.0)
        nc.vector.memset(vp_sb[:, :, 0:1], 1.0)

        # ---------- phase 1: qT / kT (64 x 8192 each, streamed) ----------
        with tc.tile_pool(name="pqk", bufs=4, space="PSUM") as pqk:
            for w_sb, dst in ((wqT_sb, qT_sb), (wkT_sb, kT_sb)):
                for n in range(T // 512):
                    ps = pqk.tile([64, 512], dt.float32)
                    for k in range(KC):
                        nc.tensor.matmul(
                            ps[:],
                            lhsT=w_sb[:, k, :],
                            rhs=xT_sb[:, k, n * 512:(n + 1) * 512],
                            start=(k == 0),
                            stop=(k == KC - 1),
                        )
                    nc.vector.tensor_copy(dst[:, n * 512:(n + 1) * 512], ps[:])

            # ---------- phase 2: v in [token, hd] layout ----------
            for t in range(T // 128):
                pv = pqk.tile([128, HD], dt.float32)
                for k in range(KC):
                    nc.tensor.matmul(
                        pv[:],
                        lhsT=xT_sb[:, k, t * 128:(t + 1) * 128],
                        rhs=wvT_sb[:, k, :],
                        start=(k == 0),
                        stop=(k == KC - 1),
                    )
                nc.vector.tensor_copy(vp_sb[:, t, 1:1 + HD], pv[:])

        # ---------- phase 3: attention ----------
        with (
            tc.tile_pool(name="pst", bufs=2, space="PSUM") as pst,
            tc.tile_pool(name="pacc", bufs=2, space="PSUM") as pacc,
        ):
            for pair in range(B):
                po = pair * N
                for qc in range(NQC):
                    qo = po + qc * QC
                    acc = pacc.tile([1 + HD, QC], dt.float32)
                    for kt in range(NKT):
                        st = pst.tile([128, QC], dt.float32)
                        lhs_k = kT_sb[:, po + kt * 128: po + (kt + 1) * 128]
                        for h in range(QC // 512):
                            nc.tensor.matmul(
                                st[:, h * 512:(h + 1) * 512],
                                lhsT=lhs_k,
                                rhs=qT_sb[:, qo + h * 512: qo + (h + 1) * 512],
                                start=True,
                                stop=True,
                            )
                        e = cpool.tile([128, QC], dt.bfloat16, tag="e")
                        nc.scalar.activation(
                            e[:], st[:], mybir.ActivationFunctionType.Exp, scale=SCALE
                        )
                        vkt = vp_sb[:, pair * NKT + kt, :]
                        for h in range(QC // 512):
                            nc.tensor.matmul(
                                acc[:, h * 512:(h + 1) * 512],
                                lhsT=vkt,
                                rhs=e[:, h * 512:(h + 1) * 512],
                                start=(kt == 0),
                                stop=(kt == NKT - 1),
                            )
                    # normalize: rows 1..64 scaled by 1/row0, row 0 becomes 1
                    recip = cpool.tile([1, QC], dt.float32, tag="recip")
                    nc.vector.reciprocal(recip[:], acc[0:1, :])
                    # partition-broadcast via a DRAM bounce (SBUF-source DMAs
                    # cannot have a zero partition step; DRAM-source can)
                    rdram = dram.tile([1, QC], dt.float32, tag="rdram")
                    nc.sync.dma_start(out=rdram[:], in_=recip[:])
                    bcast = cpool.tile([1 + HD, QC], dt.float32, tag="bcast")
                    rap = rdram[:]
                    nc.sync.dma_start(
                        out=bcast[:],
                        in_=bass.AP(
                            tensor=rap.tensor, offset=rap.offset,
                            ap=[[0, 1 + HD]] + list(rap.ap[1:]),
                        ),
                    )
                    outTn = cpool.tile([1 + HD, QC], dt.bfloat16, tag="outTn")
                    nc.vector.tensor_mul(outTn[:], acc[:], bcast[:])
                    shard = pair * NQC + qc
                    nc.sync.dma_start(out=a2a_in[shard], in_=outTn[1:1 + HD, :])

        nc.gpsimd.collective_compute(
            "AllToAll",
            mybir.AluOpType.bypass,
            replica_groups=[list(range(N_CORES))],
            ins=[a2a_in.opt()],
            outs=[a2a_out.opt()],
        )

        # ---------- phase 4: output projection on own token slice ----------
        for k in range(KC):
            nc.sync.dma_start(
                out=outTall_sb[:, k, :],
                in_=a2a_out[2 * k:2 * k + 2].rearrange("a d n -> (a d) n"),
            )
        with tc.tile_pool(name="py", bufs=2, space="PSUM") as py:
            for ts_i in range(TS // 128):
                yp = py.tile([128, D], dt.float32)
                for k in range(KC):
                    nc.tensor.matmul(
                        yp[:],
                        lhsT=outTall_sb[:, k, ts_i * 128:(ts_i + 1) * 128],
                        rhs=wpT_sb[:, k, :],
                        start=(k == 0),
                        stop=False,
                    )
                nc.tensor.matmul(
                    yp[:],
                    lhsT=ones_sb[:],
                    rhs=bias_sb[:],
                    start=False,
                    stop=True,
                )
                y_sb = cpool.tile([128, D], dt.float32, tag="y")
                nc.vector.tensor_copy(y_sb[:], yp[:])
                nc.sync.dma_start(
                    out=out_ext[ts_i * 128:(ts_i + 1) * 128, :], in_=y_sb[:]
                )

    return nc


def _get_nc():
    if "nc" not in _COMPILED:
        _COMPILED["nc"] = _build()
    return _COMPILED["nc"]


def kernel(x, w_qkv, w_proj, b_proj):
    from concourse.bass_utils import run_bass_kernel_spmd

    x = np.asarray(x, dtype=np.float32)
    w_qkv = np.asarray(w_qkv, dtype=np.float32)
    w_proj = np.asarray(w_proj, dtype=np.float32)
    b_proj = np.asarray(b_proj, dtype=np.float32)

    # host-side layout prep (bf16 compute precision on device)
    xT = np.ascontiguousarray(
        x.transpose(2, 0, 1).reshape(D, T)
    ).astype(BF16)
    wpT = np.ascontiguousarray(w_proj.T).astype(BF16)
    bias = b_proj.reshape(1, D).astype(BF16)

    in_maps = []
    for d in range(N_CORES):
        wq = w_qkv[0 * D + d * HD: 0 * D + (d + 1) * HD, :]   # [64, 512]
        wk = w_qkv[1 * D + d * HD: 1 * D + (d + 1) * HD, :]
        wv = w_qkv[2 * D + d * HD: 2 * D + (d + 1) * HD, :]
        in_maps.append({
            "xT": xT,
            "wqT": np.ascontiguousarray(wq.T).astype(BF16),
            "wkT": np.ascontiguousarray(wk.T).astype(BF16),
            "wvT": np.ascontiguousarray(wv.T).astype(BF16),
            "wpT": wpT,
            "bias": bias,
        })

    nc = _get_nc()
    res = run_bass_kernel_spmd(nc, in_maps, core_ids=list(range(N_CORES)))
    y = np.concatenate([res.results[d]["out"] for d in range(N_CORES)], axis=0)
    return y.reshape(B, N, D).astype(np.float32)
